# revision 36
# baseline (speedup 1.0000x reference)
"""NodeMPNN (message passing + GRU + LayerNorm) on 8 Trainium2 NeuronCores.

Strategy (dst-sharded graph parallel, transfer-optimized — the wall clock is
dominated by the host<->device tunnel at ~45-75MB/s, so every design choice
minimizes bytes on the wire):
  - Nodes/edges sharded by destination node across 8 cores (6250 dst/core).
  - Host ships ONE packed uint8 blob per core (~1.2MB): its node shard as
    int8 row quants + f32 row scales, compact int16 gather indices, u8 dst
    offsets, and int8 per-column-scaled weights (matmuls run in the scaled
    domain; outputs dequantize via the ACT scale operand). The node shard is
    dequantized to bf16 on-device and the full table rebuilt via an 8-core
    DRAM AllGather, so node features cross the host link once, 1 byte/elem.
  - Linearity trick: segment_sum(nodes[src] @ W^T) = segment_sum(nodes[src]) @ W^T,
    so we gather raw node rows and apply W_msg once per 512-dst block.
  - Segment sum via PE: edges sorted by dst, padded per 128-dst window;
    one-hot selection matrices built on DVE (iota is_equal against dst
    offsets; padding slots carry offset 255 so they match nothing);
    PSUM accumulates G^T @ S = messages^T per window.
  - GRU gates computed in transposed (feature-major) layout: gate = W_ih@msg^T +
    W_hh@nodes^T accumulated in PSUM; mean-node term folded into per-feature gate
    biases (partial sums AllReduced across cores); b_msg folded in as a
    rank-1 b_msg (x) deg update.
  - LayerNorm row-major after PE transposes, bn_stats/bn_aggr + ACT apply.
  - Output returned per-row int8-quantized (128 quants + f32 scale = 132B
    per node row); scales come from an on-device absmax, rounding is forced
    exact with the f32 magic-number trick, and the host dequantizes.
  - The compiled executable (jax.jit of the bass_exec custom call — the same
    lowering run_bass_kernel_spmd uses under axon) is cached across calls,
    so repeat invocations skip re-trace/re-lower/re-compile.
"""

import sys

sys.path.insert(0, "/opt/trn_rl_repo")

from contextlib import ExitStack

import numpy as np
import ml_dtypes

import concourse.bass as bass
import concourse.bacc as bacc
import concourse.tile as tile
from concourse import mybir

BF16 = ml_dtypes.bfloat16
P = 128
N_CORES = 8
WIN = 128            # dst window (one-hot width)
SB = 512             # dst super-block (PSUM free dim)
N_NODES = 50000
DIM_H = 128
SHARD = N_NODES // N_CORES        # 6250 dst nodes per core
HALF = N_NODES // 2               # lo/hi gather-table split (int16 indices)
SHARD_W = 6272                    # node rows on the wire (mult of 128)
SHARD_PAD = 6656                  # compute pad (13 super-blocks)
NSB = SHARD_PAD // SB             # 13
NW = -(-SHARD // WIN)             # 49 real dst windows per core
WPSB = SB // WIN                  # 4 windows per super-block
PAD_OFF = 255.0                   # one-hot sentinel: matches no iota column


def _host_prep(nodes, W_msg, b_msg, w_ih, w_hh, b_ih, b_hh, ln_gamma, ln_beta,
               edge_src, edge_dst):
    """Sort/pad edges, pack one uint8 blob per core + the shared schedule."""
    N, H = nodes.shape
    assert (N, H) == (N_NODES, DIM_H)

    d = np.asarray(edge_dst).astype(np.int64)
    s = np.asarray(edge_src).astype(np.int64)
    core = d // SHARD
    within = d - core * SHARD
    w_of = within // WIN
    off_of = within % WIN
    stream = (s >= HALF).astype(np.int64)
    loc = s - stream * HALF

    key = (core * NW + w_of) * 2 + stream
    order = np.argsort(key, kind="stable")
    key_s, loc_s, off_s, core_s = key[order], loc[order], off_of[order], core[order]
    w_s, st_s = w_of[order], stream[order]

    counts = np.bincount(key_s, minlength=N_CORES * NW * 2).reshape(N_CORES, NW, 2)
    tw = (counts.max(axis=0) + P - 1) // P          # [NW, 2] tiles per (window, stream)
    T = [int(tw[:, s0].sum()) for s0 in (0, 1)]
    wstart = []
    for s0 in (0, 1):
        ws = np.zeros(NW + 1, np.int64)
        ws[1:] = np.cumsum(tw[:, s0] * P)
        wstart.append(ws)

    starts_flat = np.zeros(N_CORES * NW * 2 + 1, np.int64)
    starts_flat[1:] = np.cumsum(counts.reshape(-1))
    rank = np.arange(d.shape[0], dtype=np.int64) - starts_flat[key_s]
    slot = np.where(st_s == 0, wstart[0][w_s], wstart[1][w_s]) + rank

    idx_w, dst_w = [], []
    for s0 in (0, 1):
        total = T[s0] * P
        sa = np.zeros((N_CORES, total), np.int16)
        oa = np.full((N_CORES, total), PAD_OFF, np.uint8)
        m = st_s == s0
        sa[core_s[m], slot[m]] = loc_s[m]
        oa[core_s[m], slot[m]] = off_s[m]
        # wire: indices 16-wrapped [16, T*8]; offsets slot-major [128, T] u8
        idx_w.append(np.ascontiguousarray(
            sa.reshape(N_CORES, total // 16, 16).transpose(0, 2, 1)))
        dst_w.append(np.ascontiguousarray(
            oa.reshape(N_CORES, T[s0], P).transpose(0, 2, 1)))

    has_bmsg = bool(np.any(np.asarray(b_msg, np.float32) != 0.0))
    deg_full = np.bincount(d, minlength=N).astype(np.float32)

    nodes_f32 = np.asarray(nodes, np.float32)
    wmsgT = np.ascontiguousarray(np.asarray(W_msg, np.float64).T)   # [H, H]
    wihT = np.ascontiguousarray(np.asarray(w_ih, np.float64).T)     # [H, 3H]
    whhT = np.ascontiguousarray(np.asarray(w_hh, np.float64).T)     # [H, 3H]
    bih_t = np.ascontiguousarray(np.asarray(b_ih, np.float32).reshape(3, H).T)
    bhh_t = np.ascontiguousarray(np.asarray(b_hh, np.float32).reshape(3, H).T)
    gam = np.asarray(ln_gamma, np.float32).astype(BF16)
    bet = np.asarray(ln_beta, np.float32).astype(BF16)

    # int8 weights, scaled per output column. r/z gates share one scale across
    # (w_ih, w_hh) since their matmuls accumulate into one PSUM; the n gate's
    # two halves live in separate PSUMs and get independent scales. Outputs
    # are dequantized at the existing ACT steps via their [P,1] scale APs.
    def _colq(w, s):
        return np.clip(np.round(w / s[None, :]), -127, 127).astype(np.int8)

    cmax = np.abs(np.stack([wihT, whhT])).max(axis=(0, 1))          # [3H]
    s_rz = np.maximum(cmax[:2 * H], 1e-30) / 127.0
    s_in = np.maximum(np.abs(wihT).max(axis=0)[2 * H:], 1e-30) / 127.0
    s_hn = np.maximum(np.abs(whhT).max(axis=0)[2 * H:], 1e-30) / 127.0
    s_ih = np.concatenate([s_rz, s_in])
    s_hh = np.concatenate([s_rz, s_hn])
    q_ih = _colq(wihT, s_ih)
    q_hh = _colq(whhT, s_hh)
    s_msg = np.maximum(np.abs(wmsgT).max(axis=0), 1e-30) / 127.0    # [H]
    q_msg = _colq(wmsgT, s_msg)
    # [128, 5] f32: r, z, n_ih, n_hh, msg scales (partition = output feature)
    wscl = np.ascontiguousarray(np.stack(
        [s_rz[:H], s_rz[H:], s_in, s_hn, s_msg], axis=1)).astype(np.float32)
    bmsg = (np.asarray(b_msg, np.float64) / s_msg).astype(BF16)     # scaled dom

    fields = [
        ("nodesq", SHARD_W * H),              # int8 row quants
        ("nodess", SHARD_W * 4),              # f32 row scales, [128, NCH] layout
        ("idx0", T[0] * 256), ("idx1", T[1] * 256),
        ("dst0", T[0] * 128), ("dst1", T[1] * 128),
        ("qmsg", H * H), ("qih", H * 3 * H), ("qhh", H * 3 * H),
        ("wscl", H * 5 * 4),
        ("gamma", H * 2), ("beta", H * 2),
        ("bih", H * 3 * 4), ("bhh", H * 3 * 4),
    ]
    if has_bmsg:
        fields += [("bmsg", H * 2), ("deg", SHARD_PAD * 2)]
    off, o = {}, 0
    for nm, sz in fields:
        assert sz % 4 == 0
        off[nm] = o
        o += sz
    total_bytes = o

    glob = np.zeros((N_CORES, total_bytes), np.uint8)
    for c in range(N_CORES):
        b = glob[c:c + 1]

        def put(nm, arr):
            raw = np.ascontiguousarray(arr).view(np.uint8).reshape(-1)
            b[0, off[nm]:off[nm] + raw.size] = raw

        # int8 per-row quantized node shard (pad rows: q=0, scale=0)
        nsh = np.zeros((SHARD_W, H), np.float64)
        nsh[:SHARD] = nodes_f32[c * SHARD:(c + 1) * SHARD]
        nscale = np.abs(nsh).max(axis=1) / 127.0          # [SHARD_W]
        safe = np.where(nscale > 0, nscale, 1.0)
        nq = np.clip(np.round(nsh / safe[:, None]), -127, 127).astype(np.int8)
        put("nodesq", nq)
        put("nodess", np.ascontiguousarray(
            nscale.reshape(SHARD_W // P, P).T).astype(np.float32))
        put("idx0", idx_w[0][c]); put("idx1", idx_w[1][c])
        put("dst0", dst_w[0][c]); put("dst1", dst_w[1][c])
        put("qmsg", q_msg); put("qih", q_ih); put("qhh", q_hh)
        put("wscl", wscl)
        put("gamma", gam); put("beta", bet)
        put("bih", bih_t); put("bhh", bhh_t)
        if has_bmsg:
            put("bmsg", bmsg)
            dg = np.zeros(SHARD_PAD, np.float32)
            dg[:SHARD] = deg_full[c * SHARD:(c + 1) * SHARD]
            put("deg", dg.astype(BF16))

    meta = dict(T0=T[0], T1=T[1], has_bmsg=has_bmsg,
                tw=[[int(tw[w, 0]), int(tw[w, 1])] for w in range(NW)],
                ws0=[int(x) for x in wstart[0]], ws1=[int(x) for x in wstart[1]],
                off=off, total_bytes=total_bytes)
    return glob, meta


def _build_program(meta):
    T = (meta["T0"], meta["T1"])
    tw = meta["tw"]
    wstart = (meta["ws0"], meta["ws1"])
    off = meta["off"]
    TOT = meta["total_bytes"]
    H = DIM_H

    nc = bacc.Bacc("TRN2", target_bir_lowering=False, debug=False,
                   num_devices=N_CORES)
    f32, bf16, f16 = mybir.dt.float32, mybir.dt.bfloat16, mybir.dt.float16
    i16, u8 = mybir.dt.int16, mybir.dt.uint8

    has_bmsg = meta["has_bmsg"]
    blob = nc.declare_dram_parameter("blob", [1, TOT], u8, isOutput=False)
    # packed output: int8 row quants (cols 0:128) + f32 row scale (cols 128:132)
    # full replicated output: each core AllGathers all shards so the host
    # fetches ONE contiguous buffer from a single device (fewer D2H ops)
    out_d = nc.declare_dram_parameter("out_q", [N_NODES, H + 4], mybir.dt.int8,
                                      isOutput=True)

    def view(nm, dt_, shape):
        nbytes = int(np.prod(shape)) * mybir.dt.size(dt_)
        ap = blob[0, off[nm]:off[nm] + nbytes].bitcast(dt_)
        if len(shape) == 2:
            ap = ap.rearrange("(a b) -> a b", b=shape[1])
        return ap

    with tile.TileContext(nc) as tc, ExitStack() as ctx:
        const = ctx.enter_context(tc.tile_pool(name="const", bufs=1))
        sb_g = ctx.enter_context(tc.tile_pool(name="sb_g", bufs=2))
        sb_w = ctx.enter_context(tc.tile_pool(name="sb_w", bufs=2))
        psum = ctx.enter_context(tc.tile_pool(name="psum", bufs=1, space="PSUM"))
        dram = ctx.enter_context(tc.tile_pool(name="dram", bufs=1, space="DRAM"))

        # ---- constants / parameters into SBUF ----
        i8m = mybir.dt.int8
        wmsg_t = const.tile([H, H], bf16)
        wih_t = const.tile([H, 3 * H], bf16)
        whh_t = const.tile([H, 3 * H], bf16)
        qmsg_t = const.tile([H, H], i8m)
        qih_t = const.tile([H, 3 * H], i8m)
        qhh_t = const.tile([H, 3 * H], i8m)
        wscl_sb = const.tile([H, 5], f32)
        bih_sb = const.tile([H, 3], f32)
        bhh_sb = const.tile([H, 3], f32)
        gam_row = const.tile([1, H], bf16)
        bet_row = const.tile([1, H], bf16)
        loads = [
            (qmsg_t, "qmsg", i8m, (H, H)),
            (qih_t, "qih", i8m, (H, 3 * H)),
            (qhh_t, "qhh", i8m, (H, 3 * H)),
            (wscl_sb, "wscl", f32, (H, 5)),
            (bih_sb, "bih", f32, (H, 3)),
            (bhh_sb, "bhh", f32, (H, 3)),
            (gam_row, "gamma", bf16, (1, H)),
            (bet_row, "beta", bf16, (1, H)),
        ]
        if has_bmsg:
            bmsg_row = const.tile([1, H], bf16)
            deg_row = const.tile([1, SHARD_PAD], bf16)
            loads += [(bmsg_row, "bmsg", bf16, (1, H)),
                      (deg_row, "deg", bf16, (1, SHARD_PAD))]
        dst_u8 = [const.tile([P, T[s]], u8, name=f"dstu8_t{s}") for s in (0, 1)]
        loads += [(dst_u8[0], "dst0", u8, (P, T[0])),
                  (dst_u8[1], "dst1", u8, (P, T[1]))]
        for t, nm, dt_, shp in loads:
            nc.sync.dma_start(out=t[:], in_=view(nm, dt_, shp))

        # dst offsets u8 -> bf16 (values 0..255, exact)
        dstoff_ts = [const.tile([P, T[s]], bf16, name=f"dstoff_t{s}")
                     for s in (0, 1)]
        for s in (0, 1):
            nc.vector.tensor_scalar(out=dstoff_ts[s][:], in0=dst_u8[s][:],
                                    scalar1=1.0, scalar2=None,
                                    op0=mybir.AluOpType.mult)

        # int8 weights -> bf16 in the scaled domain (values +-127, exact);
        # the per-column scales are applied at the ACT dequant steps below
        for wt, qt_ in ((wmsg_t, qmsg_t), (wih_t, qih_t), (whh_t, qhh_t)):
            nc.vector.tensor_scalar(out=wt[:], in0=qt_[:], scalar1=1.0,
                                    scalar2=None, op0=mybir.AluOpType.mult)

        # gather indices: compact [16, T*8] on the wire, replicated 8x on-chip
        idx_ts = [const.tile([P, T[s] * 8], i16, name=f"idx_t{s}") for s in (0, 1)]
        for s in (0, 1):
            iv = view(f"idx{s}", i16, (16, T[s] * 8))
            for r in range(8):
                nc.sync.dma_start(out=idx_ts[s][16 * r:16 * (r + 1), :], in_=iv)

        # iota / identity built on DVE (values 0..127, exact in bf16)
        iota_t = const.tile([P, P], bf16)
        iota_p = const.tile([P, 1], bf16)
        ident_t = const.tile([P, P], bf16)
        nc.gpsimd.iota(out=iota_t[:], pattern=[[1, P]], channel_multiplier=0,
                       allow_small_or_imprecise_dtypes=True)
        nc.gpsimd.iota(out=iota_p[:], pattern=[[0, 1]], channel_multiplier=1,
                       allow_small_or_imprecise_dtypes=True)
        iota_pb = bass.AP(tensor=iota_p.tensor, offset=iota_p.offset,
                          ap=[iota_p.ap[0], [0, P]])
        nc.vector.tensor_tensor(out=ident_t[:], in0=iota_t[:], in1=iota_pb,
                                op=mybir.AluOpType.is_equal)

        eps_t = const.tile([P, 1], f32)
        nc.vector.memset(eps_t[:], 1e-5)

        # gamma/beta broadcast across partitions via PE (ones ^T @ row)
        ones_t = const.tile([1, P], bf16)
        nc.vector.memset(ones_t[:], 1.0)
        ps_gb = psum.tile([P, 2 * H], f32, tag="ps_raw")
        nc.tensor.matmul(out=ps_gb[:, 0:H], lhsT=ones_t[:], rhs=gam_row[:],
                         start=True, stop=True)
        nc.tensor.matmul(out=ps_gb[:, H:2 * H], lhsT=ones_t[:], rhs=bet_row[:],
                         start=True, stop=True)
        gamma_sb = const.tile([P, H], f32)
        beta_sb = const.tile([P, H], f32)
        nc.scalar.copy(out=gamma_sb[:], in_=ps_gb[:, 0:H])
        nc.scalar.copy(out=beta_sb[:], in_=ps_gb[:, H:2 * H])

        # ---- dequantize int8 node shard -> bf16 in internal DRAM ----
        NCH = SHARD_W // P                      # 49 row chunks of 128
        sct = const.tile([P, NCH], f32)
        nc.sync.dma_start(out=sct[:], in_=view("nodess", f32, (P, NCH)))
        gin = dram.tile([SHARD_W, H], bf16)
        i8 = mybir.dt.int8
        qbytes = off["nodesq"]
        for cch in range(NCH):
            qv = blob[0, qbytes + cch * P * H:
                      qbytes + (cch + 1) * P * H].bitcast(i8).rearrange(
                          "(a b) -> a b", b=H)
            qt = sb_g.tile([P, H], i8, tag="dq_q", name=f"dqq{cch}")
            nc.sync.dma_start(out=qt[:], in_=qv)
            qf = sb_g.tile([P, H], f32, tag="dq_f", name=f"dqf{cch}")
            nc.vector.tensor_scalar(out=qf[:], in0=qt[:], scalar1=1.0,
                                    scalar2=None, op0=mybir.AluOpType.mult)
            col = sct[:, cch:cch + 1]
            col_b = bass.AP(tensor=col.tensor, offset=col.offset,
                            ap=[col.ap[0], [0, H]])
            xb = sb_g.tile([P, H], bf16, tag="dq_x", name=f"dqx{cch}")
            nc.vector.tensor_tensor(out=xb[:], in0=qf[:], in1=col_b,
                                    op=mybir.AluOpType.mult)
            nc.sync.dma_start(out=gin[cch * P:(cch + 1) * P], in_=xb[:])

        # local output shard staging (AllGathered into out_q at the end)
        shard_out = dram.tile([SHARD, H + 4], i8)

        # ---- full node table via on-device AllGather (rank order == shard order)
        table = dram.tile([N_NODES, H], bf16, addr_space="Shared")
        nc.gpsimd.collective_compute(
            "AllGather", mybir.AluOpType.bypass,
            replica_groups=[list(range(N_CORES))],
            ins=[gin[0:SHARD]], outs=[table[:]])
        tabs = (table[0:HALF], table[HALF:N_NODES])

        # ---- transposed node shard (resident) + mean partials ----
        nodesT = const.tile([P, SHARD_PAD], bf16)
        nc.vector.memset(nodesT[:], 0.0)
        nc.sync.dma_start(out=nodesT[:, 0:SHARD_W], in_=gin[:], transpose=True)

        part13 = const.tile([P, NSB], f32)
        nc.vector.tensor_reduce(
            out=part13[:], in_=nodesT[:].rearrange("p (s d) -> p s d", s=NSB),
            axis=mybir.AxisListType.X, op=mybir.AluOpType.add)
        musum = const.tile([P, 1], f32)
        nc.vector.tensor_reduce(out=musum[:], in_=part13[:],
                                axis=mybir.AxisListType.X, op=mybir.AluOpType.add)

        mu_in = dram.tile([P, 1], f32)
        mu_out = dram.tile([P, 1], f32, addr_space="Shared")
        nc.sync.dma_start(out=mu_in[:], in_=musum[:])
        nc.gpsimd.collective_compute(
            "AllReduce", mybir.AluOpType.add,
            replica_groups=[list(range(N_CORES))],
            ins=[mu_in[:]], outs=[mu_out[:]])
        mu_t = const.tile([P, 1], f32)
        nc.sync.dma_start(out=mu_t[:], in_=mu_out[:])
        mu_bf = const.tile([P, 1], bf16)
        nc.vector.tensor_scalar(out=mu_bf[:], in0=mu_t[:], scalar1=1.0 / N_NODES,
                                scalar2=None, op0=mybir.AluOpType.mult)

        # gate biases: biasB[:,g] = W_ih_g @ mu + b_ih_g + b_hh_g (for r,z)
        #              biasA[:,2] = W_ih_n @ mu + b_ih_n  (for n-gate tanh)
        ps_mu = psum.tile([P, 3], f32, tag="ps_r")
        for g in range(3):
            nc.tensor.matmul(out=ps_mu[:, g:g + 1], lhsT=wih_t[:, g * H:(g + 1) * H],
                             rhs=mu_bf[:], start=True, stop=True)
        mu_deq = const.tile([P, 3], f32)
        for g in range(3):       # wscl cols 0,1,2 = r, z, n_ih scales
            nc.scalar.activation(out=mu_deq[:, g:g + 1], in_=ps_mu[:, g:g + 1],
                                 func=mybir.ActivationFunctionType.Identity,
                                 scale=wscl_sb[:, g:g + 1])
        biasA = const.tile([P, 3], f32)
        biasB = const.tile([P, 3], f32)
        nc.vector.tensor_add(out=biasA[:], in0=mu_deq[:], in1=bih_sb[:])
        nc.vector.tensor_add(out=biasB[:], in0=biasA[:], in1=bhh_sb[:])

        # ---- per super-block pipeline ----
        n_tiles_s = T
        for sb in range(NSB):
            w0 = sb * WPSB
            w_end = min(w0 + WPSB, NW)

            raw_ps = psum.tile([P, SB], f32, tag="ps_raw")
            g_ts, s_ts, t_bases = [None, None], [None, None], [0, 0]
            for s in (0, 1):
                if w0 >= NW:
                    t_bases[s] = n_tiles_s[s]
                    continue
                t_bases[s] = wstart[s][w0] // P
                tsb = wstart[s][w_end] // P - t_bases[s]
                if tsb == 0:
                    continue
                g_ts[s] = sb_g.tile([P, tsb, P], bf16, tag=f"g{s}",
                                    name=f"g{s}_{sb}")
                nc.gpsimd.dma_gather(
                    out_ap=g_ts[s][:], in_ap=tabs[s],
                    idxs_ap=idx_ts[s][:, t_bases[s] * 8:(t_bases[s] + tsb) * 8],
                    num_idxs=tsb * P, num_idxs_reg=tsb * P, elem_size=H,
                    single_packet=False)
                s_ts[s] = sb_g.tile([P, tsb, P], bf16, tag=f"s{s}",
                                    name=f"s{s}_{sb}")

            for wi in range(WPSB):
                w = w0 + wi
                ntw = (tw[w][0], tw[w][1]) if w < NW else (0, 0)
                nmm = ntw[0] + ntw[1]
                if nmm == 0:
                    nc.vector.memset(raw_ps[:, wi * WIN:(wi + 1) * WIN], 0.0)
                    continue
                j = 0
                for s in (0, 1):
                    if ntw[s] == 0:
                        continue
                    wt0 = wstart[s][w] // P - t_bases[s]  # sb-local tile idx
                    # one-hot for this window/stream (DVE, broadcast APs)
                    s_sl = s_ts[s][:, wt0:wt0 + ntw[s], :]
                    dst_sl = dstoff_ts[s][:, t_bases[s] + wt0:
                                          t_bases[s] + wt0 + ntw[s]]
                    dst_b = bass.AP(tensor=dst_sl.tensor, offset=dst_sl.offset,
                                    ap=[dst_sl.ap[0], dst_sl.ap[1], [0, P]])
                    iota_b = bass.AP(tensor=iota_t.tensor, offset=iota_t.offset,
                                     ap=[iota_t.ap[0], [0, ntw[s]], iota_t.ap[1]])
                    nc.vector.tensor_tensor(out=s_sl, in0=iota_b, in1=dst_b,
                                            op=mybir.AluOpType.is_equal)
                    for k in range(ntw[s]):
                        t_loc = wt0 + k
                        nc.tensor.matmul(out=raw_ps[:, wi * WIN:(wi + 1) * WIN],
                                         lhsT=g_ts[s][:, t_loc, :],
                                         rhs=s_ts[s][:, t_loc, :],
                                         start=(j == 0), stop=(j == nmm - 1))
                        j += 1

            # messages^T = W_msg @ raw^T (+ b_msg (x) deg)
            rawT_sb = sb_w.tile([P, SB], bf16, tag="rawT")
            nc.scalar.copy(out=rawT_sb[:], in_=raw_ps[:])
            msg_ps = psum.tile([P, SB], f32, tag="ps_msg")
            nc.tensor.matmul(out=msg_ps[:], lhsT=wmsg_t[:], rhs=rawT_sb[:],
                             start=True, stop=not has_bmsg)
            if has_bmsg:
                nc.tensor.matmul(out=msg_ps[:], lhsT=bmsg_row[:],
                                 rhs=deg_row[:, sb * SB:(sb + 1) * SB],
                                 start=False, stop=True)
            msgT_sb = sb_w.tile([P, SB], bf16, tag="msgT")
            nc.scalar.activation(out=msgT_sb[:], in_=msg_ps[:],
                                 func=mybir.ActivationFunctionType.Identity,
                                 scale=wscl_sb[:, 4:5])

            # row-major messages for the final residual
            msgrow_ps = psum.tile([P, WPSB, P], bf16, tag="ps_row", bufs=2)
            for j in range(WPSB):
                nc.tensor.transpose(out=msgrow_ps[:, j, :],
                                    in_=msgT_sb[:, j * P:(j + 1) * P],
                                    identity=ident_t[:])

            # GRU gates
            nsl = nodesT[:, sb * SB:(sb + 1) * SB]
            ps_r = psum.tile([P, SB], f32, tag="ps_r")
            ps_z = psum.tile([P, SB], f32, tag="ps_z")
            ps_in = psum.tile([P, SB], f32, tag="ps_in")
            ps_hn = psum.tile([P, SB], f32, tag="ps_hn")
            nc.tensor.matmul(out=ps_r[:], lhsT=wih_t[:, 0:H], rhs=msgT_sb[:],
                             start=True, stop=False)
            nc.tensor.matmul(out=ps_r[:], lhsT=whh_t[:, 0:H], rhs=nsl,
                             start=False, stop=True)
            nc.tensor.matmul(out=ps_z[:], lhsT=wih_t[:, H:2 * H], rhs=msgT_sb[:],
                             start=True, stop=False)
            nc.tensor.matmul(out=ps_z[:], lhsT=whh_t[:, H:2 * H], rhs=nsl,
                             start=False, stop=True)
            nc.tensor.matmul(out=ps_in[:], lhsT=wih_t[:, 2 * H:3 * H],
                             rhs=msgT_sb[:], start=True, stop=True)
            nc.tensor.matmul(out=ps_hn[:], lhsT=whh_t[:, 2 * H:3 * H], rhs=nsl,
                             start=True, stop=True)

            r_sb = sb_w.tile([P, SB], bf16, tag="r")
            z_sb = sb_w.tile([P, SB], bf16, tag="z")
            hnb_sb = sb_w.tile([P, SB], bf16, tag="hnb")
            nc.scalar.activation(out=r_sb[:], in_=ps_r[:],
                                 func=mybir.ActivationFunctionType.Sigmoid,
                                 bias=biasB[:, 0:1], scale=wscl_sb[:, 0:1])
            nc.scalar.activation(out=z_sb[:], in_=ps_z[:],
                                 func=mybir.ActivationFunctionType.Sigmoid,
                                 bias=biasB[:, 1:2], scale=wscl_sb[:, 1:2])
            nc.scalar.activation(out=hnb_sb[:], in_=ps_hn[:],
                                 func=mybir.ActivationFunctionType.Identity,
                                 bias=bhh_sb[:, 2:3], scale=wscl_sb[:, 3:4])

            t_sb = sb_w.tile([P, SB], bf16, tag="t")
            nc.vector.tensor_mul(out=t_sb[:], in0=r_sb[:], in1=hnb_sb[:])
            in_sb = sb_w.tile([P, SB], f32, tag="ins")
            nc.scalar.activation(out=in_sb[:], in_=ps_in[:],
                                 func=mybir.ActivationFunctionType.Identity,
                                 scale=wscl_sb[:, 2:3])
            s2_sb = sb_w.tile([P, SB], f32, tag="s2")
            nc.vector.tensor_add(out=s2_sb[:], in0=in_sb[:], in1=t_sb[:])
            n_sb = sb_w.tile([P, SB], bf16, tag="n")
            nc.scalar.activation(out=n_sb[:], in_=s2_sb[:],
                                 func=mybir.ActivationFunctionType.Tanh,
                                 bias=biasA[:, 2:3], scale=1.0)
            d_sb = sb_w.tile([P, SB], bf16, tag="d")
            nc.vector.tensor_sub(out=d_sb[:], in0=nsl, in1=n_sb[:])
            zd_sb = sb_w.tile([P, SB], bf16, tag="zd")
            nc.vector.tensor_mul(out=zd_sb[:], in0=z_sb[:], in1=d_sb[:])
            h_sb = sb_w.tile([P, SB], bf16, tag="h")
            nc.vector.tensor_add(out=h_sb[:], in0=n_sb[:], in1=zd_sb[:])

            # transpose h to row-major
            hrow_ps = psum.tile([P, WPSB, P], bf16, tag="ps_row", bufs=2)
            for j in range(WPSB):
                nc.tensor.transpose(out=hrow_ps[:, j, :],
                                    in_=h_sb[:, j * P:(j + 1) * P],
                                    identity=ident_t[:])

            # LayerNorm over features (free axis now)
            st = sb_w.tile([P, WPSB, 6], f32, tag="st")
            mv = sb_w.tile([P, WPSB, 2], f32, tag="mv")
            for j in range(WPSB):
                nc.vector.bn_stats(out=st[:, j, :], in_=hrow_ps[:, j, :])
                nc.vector.bn_aggr(out=mv[:, j, :], in_=st[:, j, :])
            sd = sb_w.tile([P, WPSB], f32, tag="sd")
            nc.scalar.activation(out=sd[:], in_=mv[:, :, 1],
                                 func=mybir.ActivationFunctionType.Sqrt,
                                 bias=eps_t[:], scale=1.0)
            rstd = sb_w.tile([P, WPSB], f32, tag="rstd")
            nc.vector.reciprocal(out=rstd[:], in_=sd[:])
            nb = sb_w.tile([P, WPSB], f32, tag="nb")
            nc.vector.scalar_tensor_tensor(out=nb[:], in0=mv[:, :, 0], scalar=-1.0,
                                           in1=rstd[:], op0=mybir.AluOpType.mult,
                                           op1=mybir.AluOpType.mult)
            xn = sb_w.tile([P, WPSB, P], f32, tag="xn")
            for j in range(WPSB):
                nc.scalar.activation(out=xn[:, j, :], in_=hrow_ps[:, j, :],
                                     func=mybir.ActivationFunctionType.Identity,
                                     bias=nb[:, j:j + 1], scale=rstd[:, j:j + 1])

            # out = xn * gamma + beta + messages
            gam_b = bass.AP(tensor=gamma_sb.tensor, offset=gamma_sb.offset,
                            ap=[gamma_sb.ap[0], [0, WPSB], gamma_sb.ap[1]])
            bet_b = bass.AP(tensor=beta_sb.tensor, offset=beta_sb.offset,
                            ap=[beta_sb.ap[0], [0, WPSB], beta_sb.ap[1]])
            bm = sb_w.tile([P, WPSB, P], f32, tag="bm")
            nc.vector.tensor_add(out=bm[:], in0=msgrow_ps[:], in1=bet_b)
            gm = sb_w.tile([P, WPSB, P], f32, tag="gm")
            nc.vector.tensor_mul(out=gm[:], in0=xn[:], in1=gam_b)
            o32 = sb_w.tile([P, WPSB, P], f32, tag="o")
            nc.vector.tensor_add(out=o32[:], in0=gm[:], in1=bm[:])

            # per-row int8 quantization: scale = absmax/127, q = rne(out/scale)
            a32 = sb_w.tile([P, WPSB, P], f32, tag="a32")
            nc.scalar.activation(out=a32[:], in_=o32[:],
                                 func=mybir.ActivationFunctionType.Abs)
            scl = sb_w.tile([P, WPSB, 1], f32, tag="scl")
            nc.vector.tensor_reduce(out=scl[:, :, 0], in_=a32[:],
                                    axis=mybir.AxisListType.X,
                                    op=mybir.AluOpType.max)
            nc.vector.tensor_scalar(out=scl[:, :, 0], in0=scl[:, :, 0],
                                    scalar1=1e-30, scalar2=None,
                                    op0=mybir.AluOpType.add)
            nc.vector.tensor_scalar(out=scl[:, :, 0], in0=scl[:, :, 0],
                                    scalar1=1.0 / 127.0, scalar2=None,
                                    op0=mybir.AluOpType.mult)
            rscl = sb_w.tile([P, WPSB], f32, tag="rscl")
            nc.vector.reciprocal(out=rscl[:], in_=scl[:, :, 0])
            rscl_b = bass.AP(tensor=rscl.tensor, offset=rscl.offset,
                             ap=[rscl.ap[0], rscl.ap[1], [0, P]])
            tq = sb_w.tile([P, WPSB, P], f32, tag="tq")
            nc.vector.tensor_tensor(out=tq[:], in0=o32[:], in1=rscl_b,
                                    op=mybir.AluOpType.mult)
            # round-to-nearest in f32 via the 1.5*2^23 magic add (|t| <= 127),
            # so the int8 convert is exact under any convert rounding mode
            nc.vector.tensor_scalar(out=tq[:], in0=tq[:], scalar1=12582912.0,
                                    scalar2=None, op0=mybir.AluOpType.add)
            nc.vector.tensor_scalar(out=tq[:], in0=tq[:], scalar1=-12582912.0,
                                    scalar2=None, op0=mybir.AluOpType.add)
            q_i8 = sb_w.tile([P, WPSB, P], mybir.dt.int8, tag="q")
            nc.vector.tensor_scalar(out=q_i8[:], in0=tq[:], scalar1=1.0,
                                    scalar2=None, op0=mybir.AluOpType.mult)

            r0 = sb * SB
            if r0 + SB <= SHARD:
                q_rows = shard_out[r0:r0 + SB, 0:H].rearrange(
                    "(j p) f -> p j f", p=P)
                nc.sync.dma_start(out=q_rows, in_=q_i8[:])
                s_rows = shard_out[r0:r0 + SB, H:H + 4].bitcast(f32).rearrange(
                    "(j p) x -> p j x", p=P)
                nc.sync.dma_start(out=s_rows, in_=scl[:])
            elif r0 < SHARD:
                tail = SHARD - r0          # 106 rows, all within window j=0
                assert tail <= P
                nc.sync.dma_start(out=shard_out[r0:r0 + tail, 0:H],
                                  in_=q_i8[0:tail, 0, :])
                nc.sync.dma_start(
                    out=shard_out[r0:r0 + tail, H:H + 4].bitcast(f32),
                    in_=scl[0:tail, 0, :])

        # assemble the full output on every core; host fetches from one device
        out_full = dram.tile([N_NODES, H + 4], i8, addr_space="Shared")
        nc.gpsimd.collective_compute(
            "AllGather", mybir.AluOpType.bypass,
            replica_groups=[list(range(N_CORES))],
            ins=[shard_out[:]], outs=[out_full[:]])
        nc.sync.dma_start(out=out_d[:], in_=out_full[:])

    nc.finalize()
    return nc


def _meta_key(meta):
    return (meta["T0"], meta["T1"], meta["total_bytes"],
            tuple(tuple(x) for x in meta["tw"]),
            tuple(sorted(meta["off"].items())))


_RUNTIME = {}


def _get_runtime(meta):
    key = _meta_key(meta)
    if key in _RUNTIME:
        return _RUNTIME[key]

    nc = _build_program(meta)

    import jax
    from jax.sharding import Mesh, PartitionSpec
    from jax.experimental.shard_map import shard_map
    from concourse import bass2jax

    bass2jax.install_neuronx_cc_hook()
    partition_name = nc.partition_id_tensor.name if nc.partition_id_tensor else None

    in_names, out_names, out_avals = [], [], []
    for alloc in nc.m.functions[0].allocations:
        if not isinstance(alloc, mybir.MemoryLocationSet):
            continue
        name = alloc.memorylocations[0].name
        if alloc.kind == "ExternalInput":
            if name != partition_name:
                in_names.append(name)
        elif alloc.kind == "ExternalOutput":
            out_names.append(name)
            out_avals.append(jax.core.ShapedArray(
                tuple(alloc.tensor_shape), mybir.dt.np(alloc.dtype)))
    bind_in_names = tuple(in_names) + ((partition_name,) if partition_name else ())

    def _body(*args):
        operands = list(args)
        if partition_name is not None:
            operands.append(bass2jax.partition_id_tensor())
        outs = bass2jax._bass_exec_p.bind(
            *operands, out_avals=tuple(out_avals), in_names=bind_in_names,
            out_names=tuple(out_names), lowering_input_output_aliases=(),
            sim_require_finite=True, sim_require_nnan=True, nc=nc)
        return tuple(outs)

    devices = jax.devices()[:N_CORES]
    mesh = Mesh(np.asarray(devices), ("core",))
    fn = jax.jit(
        shard_map(_body, mesh=mesh,
                  in_specs=(PartitionSpec("core"),) * len(in_names),
                  out_specs=(PartitionSpec(),) * len(out_names),
                  check_rep=False),
        keep_unused=True)

    rt = (fn, nc)
    _RUNTIME[key] = rt
    return rt


def _decode(out):
    """[N, 132] int8 (q | f32 scale) -> [N, 128] f32."""
    s = out[:, DIM_H:].copy().view(np.float32)
    return np.multiply(out[:, :DIM_H], s, dtype=np.float32)


def _run(glob, rt):
    """One device round trip: upload blobs, execute, fetch + decode output.

    The output is AllGathered on-device (NeuronLink, ~0.4ms) and declared
    replicated, so this fetch is ONE contiguous 6.6MB D2H from a single
    device instead of 8 small per-shard transfers.
    """
    fn, _ = rt
    out = np.asarray(fn(glob)[0])                 # [50000, 132] int8
    return _decode(out)


def kernel(**inputs):
    glob, meta = _host_prep(**inputs)
    rt = _get_runtime(meta)
    return _run(glob, rt)


# revision 37
# speedup vs baseline: 1.2364x; 1.2364x over previous
"""NodeMPNN (message passing + GRU + LayerNorm) on 8 Trainium2 NeuronCores.

Strategy (dst-sharded graph parallel, transfer-optimized — the wall clock is
dominated by the host<->device tunnel at ~45-75MB/s, so every design choice
minimizes bytes on the wire):
  - Nodes/edges sharded by destination node across 8 cores (6250 dst/core).
  - Host ships ONE packed uint8 blob per core (~1.2MB): its node shard as
    int8 row quants + f32 row scales, compact int16 gather indices, u8 dst
    offsets, and int8 per-column-scaled weights (matmuls run in the scaled
    domain; outputs dequantize via the ACT scale operand). The node shard is
    dequantized to bf16 on-device and the full table rebuilt via an 8-core
    DRAM AllGather, so node features cross the host link once, 1 byte/elem.
  - Linearity trick: segment_sum(nodes[src] @ W^T) = segment_sum(nodes[src]) @ W^T,
    so we gather raw node rows and apply W_msg once per 512-dst block.
  - Segment sum via PE: edges sorted by dst, padded per 128-dst window;
    one-hot selection matrices built on DVE (iota is_equal against dst
    offsets; padding slots carry offset 255 so they match nothing);
    PSUM accumulates G^T @ S = messages^T per window.
  - GRU gates computed in transposed (feature-major) layout: gate = W_ih@msg^T +
    W_hh@nodes^T accumulated in PSUM; mean-node term folded into per-feature gate
    biases (partial sums AllReduced across cores); b_msg folded in as a
    rank-1 b_msg (x) deg update.
  - LayerNorm row-major after PE transposes, bn_stats/bn_aggr + ACT apply.
  - Output returned per-row int8-quantized (128 quants + f32 scale = 132B
    per node row); scales come from an on-device absmax, rounding is forced
    exact with the f32 magic-number trick, and the host dequantizes.
  - The compiled executable (jax.jit of the bass_exec custom call — the same
    lowering run_bass_kernel_spmd uses under axon) is cached across calls,
    so repeat invocations skip re-trace/re-lower/re-compile.
"""

import sys

sys.path.insert(0, "/opt/trn_rl_repo")

from contextlib import ExitStack

import numpy as np
import ml_dtypes

import concourse.bass as bass
import concourse.bacc as bacc
import concourse.tile as tile
from concourse import mybir

BF16 = ml_dtypes.bfloat16
P = 128
N_CORES = 8
WIN = 128            # dst window (one-hot width)
SB = 512             # dst super-block (PSUM free dim)
N_NODES = 50000
DIM_H = 128
SHARD = N_NODES // N_CORES        # 6250 dst nodes per core
HALF = N_NODES // 2               # lo/hi gather-table split (int16 indices)
SHARD_W = 6272                    # node rows on the wire (mult of 128)
SHARD_PAD = 6656                  # compute pad (13 super-blocks)
NSB = SHARD_PAD // SB             # 13
NW = -(-SHARD // WIN)             # 49 real dst windows per core
WPSB = SB // WIN                  # 4 windows per super-block
PAD_OFF = 255.0                   # one-hot sentinel: matches no iota column


def _host_prep(nodes, W_msg, b_msg, w_ih, w_hh, b_ih, b_hh, ln_gamma, ln_beta,
               edge_src, edge_dst):
    """Sort/pad edges, pack one uint8 blob per core + the shared schedule."""
    N, H = nodes.shape
    assert (N, H) == (N_NODES, DIM_H)

    d = np.asarray(edge_dst).astype(np.int64)
    s = np.asarray(edge_src).astype(np.int64)
    core = d // SHARD
    within = d - core * SHARD
    w_of = within // WIN
    off_of = within % WIN
    stream = (s >= HALF).astype(np.int64)
    loc = s - stream * HALF

    key = (core * NW + w_of) * 2 + stream
    order = np.argsort(key, kind="stable")
    key_s, loc_s, off_s, core_s = key[order], loc[order], off_of[order], core[order]
    w_s, st_s = w_of[order], stream[order]

    counts = np.bincount(key_s, minlength=N_CORES * NW * 2).reshape(N_CORES, NW, 2)
    tw = (counts.max(axis=0) + P - 1) // P          # [NW, 2] tiles per (window, stream)
    T = [int(tw[:, s0].sum()) for s0 in (0, 1)]
    wstart = []
    for s0 in (0, 1):
        ws = np.zeros(NW + 1, np.int64)
        ws[1:] = np.cumsum(tw[:, s0] * P)
        wstart.append(ws)

    starts_flat = np.zeros(N_CORES * NW * 2 + 1, np.int64)
    starts_flat[1:] = np.cumsum(counts.reshape(-1))
    rank = np.arange(d.shape[0], dtype=np.int64) - starts_flat[key_s]
    slot = np.where(st_s == 0, wstart[0][w_s], wstart[1][w_s]) + rank

    idx_w, dst_w = [], []
    for s0 in (0, 1):
        total = T[s0] * P
        sa = np.zeros((N_CORES, total), np.int16)
        oa = np.full((N_CORES, total), PAD_OFF, np.uint8)
        m = st_s == s0
        sa[core_s[m], slot[m]] = loc_s[m]
        oa[core_s[m], slot[m]] = off_s[m]
        # wire: indices 16-wrapped [16, T*8]; offsets slot-major [128, T] u8
        idx_w.append(np.ascontiguousarray(
            sa.reshape(N_CORES, total // 16, 16).transpose(0, 2, 1)))
        dst_w.append(np.ascontiguousarray(
            oa.reshape(N_CORES, T[s0], P).transpose(0, 2, 1)))

    has_bmsg = bool(np.any(np.asarray(b_msg, np.float32) != 0.0))
    deg_full = np.bincount(d, minlength=N).astype(np.float32)

    nodes_f32 = np.asarray(nodes, np.float32)
    wmsgT = np.ascontiguousarray(np.asarray(W_msg, np.float64).T)   # [H, H]
    wihT = np.ascontiguousarray(np.asarray(w_ih, np.float64).T)     # [H, 3H]
    whhT = np.ascontiguousarray(np.asarray(w_hh, np.float64).T)     # [H, 3H]
    bih_t = np.ascontiguousarray(np.asarray(b_ih, np.float32).reshape(3, H).T)
    bhh_t = np.ascontiguousarray(np.asarray(b_hh, np.float32).reshape(3, H).T)
    gam = np.asarray(ln_gamma, np.float32).astype(BF16)
    bet = np.asarray(ln_beta, np.float32).astype(BF16)

    # int8 weights, scaled per output column. r/z gates share one scale across
    # (w_ih, w_hh) since their matmuls accumulate into one PSUM; the n gate's
    # two halves live in separate PSUMs and get independent scales. Outputs
    # are dequantized at the existing ACT steps via their [P,1] scale APs.
    def _colq(w, s):
        return np.clip(np.round(w / s[None, :]), -127, 127).astype(np.int8)

    cmax = np.abs(np.stack([wihT, whhT])).max(axis=(0, 1))          # [3H]
    s_rz = np.maximum(cmax[:2 * H], 1e-30) / 127.0
    s_in = np.maximum(np.abs(wihT).max(axis=0)[2 * H:], 1e-30) / 127.0
    s_hn = np.maximum(np.abs(whhT).max(axis=0)[2 * H:], 1e-30) / 127.0
    s_ih = np.concatenate([s_rz, s_in])
    s_hh = np.concatenate([s_rz, s_hn])
    q_ih = _colq(wihT, s_ih)
    q_hh = _colq(whhT, s_hh)
    s_msg = np.maximum(np.abs(wmsgT).max(axis=0), 1e-30) / 127.0    # [H]
    q_msg = _colq(wmsgT, s_msg)
    # [128, 5] f32: r, z, n_ih, n_hh, msg scales (partition = output feature)
    wscl = np.ascontiguousarray(np.stack(
        [s_rz[:H], s_rz[H:], s_in, s_hn, s_msg], axis=1)).astype(np.float32)
    bmsg = (np.asarray(b_msg, np.float64) / s_msg).astype(BF16)     # scaled dom

    fields = [
        ("nodesq", SHARD_W * H),              # int8 row quants
        ("nodess", SHARD_W * 4),              # f32 row scales, [128, NCH] layout
        ("idx0", T[0] * 256), ("idx1", T[1] * 256),
        ("dst0", T[0] * 128), ("dst1", T[1] * 128),
        ("qmsg", H * H), ("qih", H * 3 * H), ("qhh", H * 3 * H),
        ("wscl", H * 5 * 4),
        ("gamma", H * 2), ("beta", H * 2),
        ("bih", H * 3 * 4), ("bhh", H * 3 * 4),
    ]
    if has_bmsg:
        fields += [("bmsg", H * 2), ("deg", SHARD_PAD * 2)]
    off, o = {}, 0
    for nm, sz in fields:
        assert sz % 4 == 0
        off[nm] = o
        o += sz
    total_bytes = o

    glob = np.zeros((N_CORES, total_bytes), np.uint8)
    for c in range(N_CORES):
        b = glob[c:c + 1]

        def put(nm, arr):
            raw = np.ascontiguousarray(arr).view(np.uint8).reshape(-1)
            b[0, off[nm]:off[nm] + raw.size] = raw

        # int8 per-row quantized node shard (pad rows: q=0, scale=0)
        nsh = np.zeros((SHARD_W, H), np.float64)
        nsh[:SHARD] = nodes_f32[c * SHARD:(c + 1) * SHARD]
        nscale = np.abs(nsh).max(axis=1) / 127.0          # [SHARD_W]
        safe = np.where(nscale > 0, nscale, 1.0)
        nq = np.clip(np.round(nsh / safe[:, None]), -127, 127).astype(np.int8)
        put("nodesq", nq)
        put("nodess", np.ascontiguousarray(
            nscale.reshape(SHARD_W // P, P).T).astype(np.float32))
        put("idx0", idx_w[0][c]); put("idx1", idx_w[1][c])
        put("dst0", dst_w[0][c]); put("dst1", dst_w[1][c])
        put("qmsg", q_msg); put("qih", q_ih); put("qhh", q_hh)
        put("wscl", wscl)
        put("gamma", gam); put("beta", bet)
        put("bih", bih_t); put("bhh", bhh_t)
        if has_bmsg:
            put("bmsg", bmsg)
            dg = np.zeros(SHARD_PAD, np.float32)
            dg[:SHARD] = deg_full[c * SHARD:(c + 1) * SHARD]
            put("deg", dg.astype(BF16))

    meta = dict(T0=T[0], T1=T[1], has_bmsg=has_bmsg,
                tw=[[int(tw[w, 0]), int(tw[w, 1])] for w in range(NW)],
                ws0=[int(x) for x in wstart[0]], ws1=[int(x) for x in wstart[1]],
                off=off, total_bytes=total_bytes)
    return glob, meta


def _build_program(meta):
    T = (meta["T0"], meta["T1"])
    tw = meta["tw"]
    wstart = (meta["ws0"], meta["ws1"])
    off = meta["off"]
    TOT = meta["total_bytes"]
    H = DIM_H

    nc = bacc.Bacc("TRN2", target_bir_lowering=False, debug=False,
                   num_devices=N_CORES)
    f32, bf16, f16 = mybir.dt.float32, mybir.dt.bfloat16, mybir.dt.float16
    i16, u8 = mybir.dt.int16, mybir.dt.uint8

    has_bmsg = meta["has_bmsg"]
    blob = nc.declare_dram_parameter("blob", [1, TOT], u8, isOutput=False)
    # packed output: int8 row quants (cols 0:128) + f32 row scale (cols 128:132)
    out_d = nc.declare_dram_parameter("out_q", [SHARD, H + 4], mybir.dt.int8,
                                      isOutput=True)

    def view(nm, dt_, shape):
        nbytes = int(np.prod(shape)) * mybir.dt.size(dt_)
        ap = blob[0, off[nm]:off[nm] + nbytes].bitcast(dt_)
        if len(shape) == 2:
            ap = ap.rearrange("(a b) -> a b", b=shape[1])
        return ap

    with tile.TileContext(nc) as tc, ExitStack() as ctx:
        const = ctx.enter_context(tc.tile_pool(name="const", bufs=1))
        sb_g = ctx.enter_context(tc.tile_pool(name="sb_g", bufs=2))
        sb_w = ctx.enter_context(tc.tile_pool(name="sb_w", bufs=2))
        psum = ctx.enter_context(tc.tile_pool(name="psum", bufs=1, space="PSUM"))
        dram = ctx.enter_context(tc.tile_pool(name="dram", bufs=1, space="DRAM"))

        # ---- constants / parameters into SBUF ----
        i8m = mybir.dt.int8
        wmsg_t = const.tile([H, H], bf16)
        wih_t = const.tile([H, 3 * H], bf16)
        whh_t = const.tile([H, 3 * H], bf16)
        qmsg_t = const.tile([H, H], i8m)
        qih_t = const.tile([H, 3 * H], i8m)
        qhh_t = const.tile([H, 3 * H], i8m)
        wscl_sb = const.tile([H, 5], f32)
        bih_sb = const.tile([H, 3], f32)
        bhh_sb = const.tile([H, 3], f32)
        gam_row = const.tile([1, H], bf16)
        bet_row = const.tile([1, H], bf16)
        loads = [
            (qmsg_t, "qmsg", i8m, (H, H)),
            (qih_t, "qih", i8m, (H, 3 * H)),
            (qhh_t, "qhh", i8m, (H, 3 * H)),
            (wscl_sb, "wscl", f32, (H, 5)),
            (bih_sb, "bih", f32, (H, 3)),
            (bhh_sb, "bhh", f32, (H, 3)),
            (gam_row, "gamma", bf16, (1, H)),
            (bet_row, "beta", bf16, (1, H)),
        ]
        if has_bmsg:
            bmsg_row = const.tile([1, H], bf16)
            deg_row = const.tile([1, SHARD_PAD], bf16)
            loads += [(bmsg_row, "bmsg", bf16, (1, H)),
                      (deg_row, "deg", bf16, (1, SHARD_PAD))]
        dst_u8 = [const.tile([P, T[s]], u8, name=f"dstu8_t{s}") for s in (0, 1)]
        loads += [(dst_u8[0], "dst0", u8, (P, T[0])),
                  (dst_u8[1], "dst1", u8, (P, T[1]))]
        for t, nm, dt_, shp in loads:
            nc.sync.dma_start(out=t[:], in_=view(nm, dt_, shp))

        # dst offsets u8 -> bf16 (values 0..255, exact)
        dstoff_ts = [const.tile([P, T[s]], bf16, name=f"dstoff_t{s}")
                     for s in (0, 1)]
        for s in (0, 1):
            nc.vector.tensor_scalar(out=dstoff_ts[s][:], in0=dst_u8[s][:],
                                    scalar1=1.0, scalar2=None,
                                    op0=mybir.AluOpType.mult)

        # int8 weights -> bf16 in the scaled domain (values +-127, exact);
        # the per-column scales are applied at the ACT dequant steps below
        for wt, qt_ in ((wmsg_t, qmsg_t), (wih_t, qih_t), (whh_t, qhh_t)):
            nc.vector.tensor_scalar(out=wt[:], in0=qt_[:], scalar1=1.0,
                                    scalar2=None, op0=mybir.AluOpType.mult)

        # gather indices: compact [16, T*8] on the wire, replicated 8x on-chip
        idx_ts = [const.tile([P, T[s] * 8], i16, name=f"idx_t{s}") for s in (0, 1)]
        for s in (0, 1):
            iv = view(f"idx{s}", i16, (16, T[s] * 8))
            for r in range(8):
                nc.sync.dma_start(out=idx_ts[s][16 * r:16 * (r + 1), :], in_=iv)

        # iota / identity built on DVE (values 0..127, exact in bf16)
        iota_t = const.tile([P, P], bf16)
        iota_p = const.tile([P, 1], bf16)
        ident_t = const.tile([P, P], bf16)
        nc.gpsimd.iota(out=iota_t[:], pattern=[[1, P]], channel_multiplier=0,
                       allow_small_or_imprecise_dtypes=True)
        nc.gpsimd.iota(out=iota_p[:], pattern=[[0, 1]], channel_multiplier=1,
                       allow_small_or_imprecise_dtypes=True)
        iota_pb = bass.AP(tensor=iota_p.tensor, offset=iota_p.offset,
                          ap=[iota_p.ap[0], [0, P]])
        nc.vector.tensor_tensor(out=ident_t[:], in0=iota_t[:], in1=iota_pb,
                                op=mybir.AluOpType.is_equal)

        eps_t = const.tile([P, 1], f32)
        nc.vector.memset(eps_t[:], 1e-5)

        # gamma/beta broadcast across partitions via PE (ones ^T @ row)
        ones_t = const.tile([1, P], bf16)
        nc.vector.memset(ones_t[:], 1.0)
        ps_gb = psum.tile([P, 2 * H], f32, tag="ps_raw")
        nc.tensor.matmul(out=ps_gb[:, 0:H], lhsT=ones_t[:], rhs=gam_row[:],
                         start=True, stop=True)
        nc.tensor.matmul(out=ps_gb[:, H:2 * H], lhsT=ones_t[:], rhs=bet_row[:],
                         start=True, stop=True)
        gamma_sb = const.tile([P, H], f32)
        beta_sb = const.tile([P, H], f32)
        nc.scalar.copy(out=gamma_sb[:], in_=ps_gb[:, 0:H])
        nc.scalar.copy(out=beta_sb[:], in_=ps_gb[:, H:2 * H])

        # ---- dequantize int8 node shard -> bf16 in internal DRAM ----
        NCH = SHARD_W // P                      # 49 row chunks of 128
        sct = const.tile([P, NCH], f32)
        nc.sync.dma_start(out=sct[:], in_=view("nodess", f32, (P, NCH)))
        gin = dram.tile([SHARD_W, H], bf16)
        i8 = mybir.dt.int8
        qbytes = off["nodesq"]
        for cch in range(NCH):
            qv = blob[0, qbytes + cch * P * H:
                      qbytes + (cch + 1) * P * H].bitcast(i8).rearrange(
                          "(a b) -> a b", b=H)
            qt = sb_g.tile([P, H], i8, tag="dq_q", name=f"dqq{cch}")
            nc.sync.dma_start(out=qt[:], in_=qv)
            qf = sb_g.tile([P, H], f32, tag="dq_f", name=f"dqf{cch}")
            nc.vector.tensor_scalar(out=qf[:], in0=qt[:], scalar1=1.0,
                                    scalar2=None, op0=mybir.AluOpType.mult)
            col = sct[:, cch:cch + 1]
            col_b = bass.AP(tensor=col.tensor, offset=col.offset,
                            ap=[col.ap[0], [0, H]])
            xb = sb_g.tile([P, H], bf16, tag="dq_x", name=f"dqx{cch}")
            nc.vector.tensor_tensor(out=xb[:], in0=qf[:], in1=col_b,
                                    op=mybir.AluOpType.mult)
            nc.sync.dma_start(out=gin[cch * P:(cch + 1) * P], in_=xb[:])

        # ---- full node table via on-device AllGather (rank order == shard order)
        table = dram.tile([N_NODES, H], bf16, addr_space="Shared")
        nc.gpsimd.collective_compute(
            "AllGather", mybir.AluOpType.bypass,
            replica_groups=[list(range(N_CORES))],
            ins=[gin[0:SHARD]], outs=[table[:]])
        tabs = (table[0:HALF], table[HALF:N_NODES])

        # ---- transposed node shard (resident) + mean partials ----
        nodesT = const.tile([P, SHARD_PAD], bf16)
        nc.vector.memset(nodesT[:], 0.0)
        nc.sync.dma_start(out=nodesT[:, 0:SHARD_W], in_=gin[:], transpose=True)

        part13 = const.tile([P, NSB], f32)
        nc.vector.tensor_reduce(
            out=part13[:], in_=nodesT[:].rearrange("p (s d) -> p s d", s=NSB),
            axis=mybir.AxisListType.X, op=mybir.AluOpType.add)
        musum = const.tile([P, 1], f32)
        nc.vector.tensor_reduce(out=musum[:], in_=part13[:],
                                axis=mybir.AxisListType.X, op=mybir.AluOpType.add)

        mu_in = dram.tile([P, 1], f32)
        mu_out = dram.tile([P, 1], f32, addr_space="Shared")
        nc.sync.dma_start(out=mu_in[:], in_=musum[:])
        nc.gpsimd.collective_compute(
            "AllReduce", mybir.AluOpType.add,
            replica_groups=[list(range(N_CORES))],
            ins=[mu_in[:]], outs=[mu_out[:]])
        mu_t = const.tile([P, 1], f32)
        nc.sync.dma_start(out=mu_t[:], in_=mu_out[:])
        mu_bf = const.tile([P, 1], bf16)
        nc.vector.tensor_scalar(out=mu_bf[:], in0=mu_t[:], scalar1=1.0 / N_NODES,
                                scalar2=None, op0=mybir.AluOpType.mult)

        # gate biases: biasB[:,g] = W_ih_g @ mu + b_ih_g + b_hh_g (for r,z)
        #              biasA[:,2] = W_ih_n @ mu + b_ih_n  (for n-gate tanh)
        ps_mu = psum.tile([P, 3], f32, tag="ps_r")
        for g in range(3):
            nc.tensor.matmul(out=ps_mu[:, g:g + 1], lhsT=wih_t[:, g * H:(g + 1) * H],
                             rhs=mu_bf[:], start=True, stop=True)
        mu_deq = const.tile([P, 3], f32)
        for g in range(3):       # wscl cols 0,1,2 = r, z, n_ih scales
            nc.scalar.activation(out=mu_deq[:, g:g + 1], in_=ps_mu[:, g:g + 1],
                                 func=mybir.ActivationFunctionType.Identity,
                                 scale=wscl_sb[:, g:g + 1])
        biasA = const.tile([P, 3], f32)
        biasB = const.tile([P, 3], f32)
        nc.vector.tensor_add(out=biasA[:], in0=mu_deq[:], in1=bih_sb[:])
        nc.vector.tensor_add(out=biasB[:], in0=biasA[:], in1=bhh_sb[:])

        # ---- per super-block pipeline ----
        n_tiles_s = T
        for sb in range(NSB):
            w0 = sb * WPSB
            w_end = min(w0 + WPSB, NW)

            raw_ps = psum.tile([P, SB], f32, tag="ps_raw")
            g_ts, s_ts, t_bases = [None, None], [None, None], [0, 0]
            for s in (0, 1):
                if w0 >= NW:
                    t_bases[s] = n_tiles_s[s]
                    continue
                t_bases[s] = wstart[s][w0] // P
                tsb = wstart[s][w_end] // P - t_bases[s]
                if tsb == 0:
                    continue
                g_ts[s] = sb_g.tile([P, tsb, P], bf16, tag=f"g{s}",
                                    name=f"g{s}_{sb}")
                nc.gpsimd.dma_gather(
                    out_ap=g_ts[s][:], in_ap=tabs[s],
                    idxs_ap=idx_ts[s][:, t_bases[s] * 8:(t_bases[s] + tsb) * 8],
                    num_idxs=tsb * P, num_idxs_reg=tsb * P, elem_size=H,
                    single_packet=False)
                s_ts[s] = sb_g.tile([P, tsb, P], bf16, tag=f"s{s}",
                                    name=f"s{s}_{sb}")

            for wi in range(WPSB):
                w = w0 + wi
                ntw = (tw[w][0], tw[w][1]) if w < NW else (0, 0)
                nmm = ntw[0] + ntw[1]
                if nmm == 0:
                    nc.vector.memset(raw_ps[:, wi * WIN:(wi + 1) * WIN], 0.0)
                    continue
                j = 0
                for s in (0, 1):
                    if ntw[s] == 0:
                        continue
                    wt0 = wstart[s][w] // P - t_bases[s]  # sb-local tile idx
                    # one-hot for this window/stream (DVE, broadcast APs)
                    s_sl = s_ts[s][:, wt0:wt0 + ntw[s], :]
                    dst_sl = dstoff_ts[s][:, t_bases[s] + wt0:
                                          t_bases[s] + wt0 + ntw[s]]
                    dst_b = bass.AP(tensor=dst_sl.tensor, offset=dst_sl.offset,
                                    ap=[dst_sl.ap[0], dst_sl.ap[1], [0, P]])
                    iota_b = bass.AP(tensor=iota_t.tensor, offset=iota_t.offset,
                                     ap=[iota_t.ap[0], [0, ntw[s]], iota_t.ap[1]])
                    nc.vector.tensor_tensor(out=s_sl, in0=iota_b, in1=dst_b,
                                            op=mybir.AluOpType.is_equal)
                    for k in range(ntw[s]):
                        t_loc = wt0 + k
                        nc.tensor.matmul(out=raw_ps[:, wi * WIN:(wi + 1) * WIN],
                                         lhsT=g_ts[s][:, t_loc, :],
                                         rhs=s_ts[s][:, t_loc, :],
                                         start=(j == 0), stop=(j == nmm - 1))
                        j += 1

            # messages^T = W_msg @ raw^T (+ b_msg (x) deg)
            rawT_sb = sb_w.tile([P, SB], bf16, tag="rawT")
            nc.scalar.copy(out=rawT_sb[:], in_=raw_ps[:])
            msg_ps = psum.tile([P, SB], f32, tag="ps_msg")
            nc.tensor.matmul(out=msg_ps[:], lhsT=wmsg_t[:], rhs=rawT_sb[:],
                             start=True, stop=not has_bmsg)
            if has_bmsg:
                nc.tensor.matmul(out=msg_ps[:], lhsT=bmsg_row[:],
                                 rhs=deg_row[:, sb * SB:(sb + 1) * SB],
                                 start=False, stop=True)
            msgT_sb = sb_w.tile([P, SB], bf16, tag="msgT")
            nc.scalar.activation(out=msgT_sb[:], in_=msg_ps[:],
                                 func=mybir.ActivationFunctionType.Identity,
                                 scale=wscl_sb[:, 4:5])

            # row-major messages for the final residual
            msgrow_ps = psum.tile([P, WPSB, P], bf16, tag="ps_row", bufs=2)
            for j in range(WPSB):
                nc.tensor.transpose(out=msgrow_ps[:, j, :],
                                    in_=msgT_sb[:, j * P:(j + 1) * P],
                                    identity=ident_t[:])

            # GRU gates
            nsl = nodesT[:, sb * SB:(sb + 1) * SB]
            ps_r = psum.tile([P, SB], f32, tag="ps_r")
            ps_z = psum.tile([P, SB], f32, tag="ps_z")
            ps_in = psum.tile([P, SB], f32, tag="ps_in")
            ps_hn = psum.tile([P, SB], f32, tag="ps_hn")
            nc.tensor.matmul(out=ps_r[:], lhsT=wih_t[:, 0:H], rhs=msgT_sb[:],
                             start=True, stop=False)
            nc.tensor.matmul(out=ps_r[:], lhsT=whh_t[:, 0:H], rhs=nsl,
                             start=False, stop=True)
            nc.tensor.matmul(out=ps_z[:], lhsT=wih_t[:, H:2 * H], rhs=msgT_sb[:],
                             start=True, stop=False)
            nc.tensor.matmul(out=ps_z[:], lhsT=whh_t[:, H:2 * H], rhs=nsl,
                             start=False, stop=True)
            nc.tensor.matmul(out=ps_in[:], lhsT=wih_t[:, 2 * H:3 * H],
                             rhs=msgT_sb[:], start=True, stop=True)
            nc.tensor.matmul(out=ps_hn[:], lhsT=whh_t[:, 2 * H:3 * H], rhs=nsl,
                             start=True, stop=True)

            r_sb = sb_w.tile([P, SB], bf16, tag="r")
            z_sb = sb_w.tile([P, SB], bf16, tag="z")
            hnb_sb = sb_w.tile([P, SB], bf16, tag="hnb")
            nc.scalar.activation(out=r_sb[:], in_=ps_r[:],
                                 func=mybir.ActivationFunctionType.Sigmoid,
                                 bias=biasB[:, 0:1], scale=wscl_sb[:, 0:1])
            nc.scalar.activation(out=z_sb[:], in_=ps_z[:],
                                 func=mybir.ActivationFunctionType.Sigmoid,
                                 bias=biasB[:, 1:2], scale=wscl_sb[:, 1:2])
            nc.scalar.activation(out=hnb_sb[:], in_=ps_hn[:],
                                 func=mybir.ActivationFunctionType.Identity,
                                 bias=bhh_sb[:, 2:3], scale=wscl_sb[:, 3:4])

            t_sb = sb_w.tile([P, SB], bf16, tag="t")
            nc.vector.tensor_mul(out=t_sb[:], in0=r_sb[:], in1=hnb_sb[:])
            in_sb = sb_w.tile([P, SB], f32, tag="ins")
            nc.scalar.activation(out=in_sb[:], in_=ps_in[:],
                                 func=mybir.ActivationFunctionType.Identity,
                                 scale=wscl_sb[:, 2:3])
            s2_sb = sb_w.tile([P, SB], f32, tag="s2")
            nc.vector.tensor_add(out=s2_sb[:], in0=in_sb[:], in1=t_sb[:])
            n_sb = sb_w.tile([P, SB], bf16, tag="n")
            nc.scalar.activation(out=n_sb[:], in_=s2_sb[:],
                                 func=mybir.ActivationFunctionType.Tanh,
                                 bias=biasA[:, 2:3], scale=1.0)
            d_sb = sb_w.tile([P, SB], bf16, tag="d")
            nc.vector.tensor_sub(out=d_sb[:], in0=nsl, in1=n_sb[:])
            zd_sb = sb_w.tile([P, SB], bf16, tag="zd")
            nc.vector.tensor_mul(out=zd_sb[:], in0=z_sb[:], in1=d_sb[:])
            h_sb = sb_w.tile([P, SB], bf16, tag="h")
            nc.vector.tensor_add(out=h_sb[:], in0=n_sb[:], in1=zd_sb[:])

            # transpose h to row-major
            hrow_ps = psum.tile([P, WPSB, P], bf16, tag="ps_row", bufs=2)
            for j in range(WPSB):
                nc.tensor.transpose(out=hrow_ps[:, j, :],
                                    in_=h_sb[:, j * P:(j + 1) * P],
                                    identity=ident_t[:])

            # LayerNorm over features (free axis now)
            st = sb_w.tile([P, WPSB, 6], f32, tag="st")
            mv = sb_w.tile([P, WPSB, 2], f32, tag="mv")
            for j in range(WPSB):
                nc.vector.bn_stats(out=st[:, j, :], in_=hrow_ps[:, j, :])
                nc.vector.bn_aggr(out=mv[:, j, :], in_=st[:, j, :])
            sd = sb_w.tile([P, WPSB], f32, tag="sd")
            nc.scalar.activation(out=sd[:], in_=mv[:, :, 1],
                                 func=mybir.ActivationFunctionType.Sqrt,
                                 bias=eps_t[:], scale=1.0)
            rstd = sb_w.tile([P, WPSB], f32, tag="rstd")
            nc.vector.reciprocal(out=rstd[:], in_=sd[:])
            nb = sb_w.tile([P, WPSB], f32, tag="nb")
            nc.vector.scalar_tensor_tensor(out=nb[:], in0=mv[:, :, 0], scalar=-1.0,
                                           in1=rstd[:], op0=mybir.AluOpType.mult,
                                           op1=mybir.AluOpType.mult)
            xn = sb_w.tile([P, WPSB, P], f32, tag="xn")
            for j in range(WPSB):
                nc.scalar.activation(out=xn[:, j, :], in_=hrow_ps[:, j, :],
                                     func=mybir.ActivationFunctionType.Identity,
                                     bias=nb[:, j:j + 1], scale=rstd[:, j:j + 1])

            # out = xn * gamma + beta + messages
            gam_b = bass.AP(tensor=gamma_sb.tensor, offset=gamma_sb.offset,
                            ap=[gamma_sb.ap[0], [0, WPSB], gamma_sb.ap[1]])
            bet_b = bass.AP(tensor=beta_sb.tensor, offset=beta_sb.offset,
                            ap=[beta_sb.ap[0], [0, WPSB], beta_sb.ap[1]])
            bm = sb_w.tile([P, WPSB, P], f32, tag="bm")
            nc.vector.tensor_add(out=bm[:], in0=msgrow_ps[:], in1=bet_b)
            gm = sb_w.tile([P, WPSB, P], f32, tag="gm")
            nc.vector.tensor_mul(out=gm[:], in0=xn[:], in1=gam_b)
            o32 = sb_w.tile([P, WPSB, P], f32, tag="o")
            nc.vector.tensor_add(out=o32[:], in0=gm[:], in1=bm[:])

            # per-row int8 quantization: scale = absmax/127, q = rne(out/scale)
            a32 = sb_w.tile([P, WPSB, P], f32, tag="a32")
            nc.scalar.activation(out=a32[:], in_=o32[:],
                                 func=mybir.ActivationFunctionType.Abs)
            scl = sb_w.tile([P, WPSB, 1], f32, tag="scl")
            nc.vector.tensor_reduce(out=scl[:, :, 0], in_=a32[:],
                                    axis=mybir.AxisListType.X,
                                    op=mybir.AluOpType.max)
            nc.vector.tensor_scalar(out=scl[:, :, 0], in0=scl[:, :, 0],
                                    scalar1=1e-30, scalar2=None,
                                    op0=mybir.AluOpType.add)
            nc.vector.tensor_scalar(out=scl[:, :, 0], in0=scl[:, :, 0],
                                    scalar1=1.0 / 127.0, scalar2=None,
                                    op0=mybir.AluOpType.mult)
            rscl = sb_w.tile([P, WPSB], f32, tag="rscl")
            nc.vector.reciprocal(out=rscl[:], in_=scl[:, :, 0])
            rscl_b = bass.AP(tensor=rscl.tensor, offset=rscl.offset,
                             ap=[rscl.ap[0], rscl.ap[1], [0, P]])
            tq = sb_w.tile([P, WPSB, P], f32, tag="tq")
            nc.vector.tensor_tensor(out=tq[:], in0=o32[:], in1=rscl_b,
                                    op=mybir.AluOpType.mult)
            # round-to-nearest in f32 via the 1.5*2^23 magic add (|t| <= 127),
            # so the int8 convert is exact under any convert rounding mode
            nc.vector.tensor_scalar(out=tq[:], in0=tq[:], scalar1=12582912.0,
                                    scalar2=None, op0=mybir.AluOpType.add)
            nc.vector.tensor_scalar(out=tq[:], in0=tq[:], scalar1=-12582912.0,
                                    scalar2=None, op0=mybir.AluOpType.add)
            q_i8 = sb_w.tile([P, WPSB, P], mybir.dt.int8, tag="q")
            nc.vector.tensor_scalar(out=q_i8[:], in0=tq[:], scalar1=1.0,
                                    scalar2=None, op0=mybir.AluOpType.mult)

            r0 = sb * SB
            if r0 + SB <= SHARD:
                q_rows = out_d[r0:r0 + SB, 0:H].rearrange("(j p) f -> p j f", p=P)
                nc.sync.dma_start(out=q_rows, in_=q_i8[:])
                s_rows = out_d[r0:r0 + SB, H:H + 4].bitcast(f32).rearrange(
                    "(j p) x -> p j x", p=P)
                nc.sync.dma_start(out=s_rows, in_=scl[:])
            elif r0 < SHARD:
                tail = SHARD - r0          # 106 rows, all within window j=0
                assert tail <= P
                nc.sync.dma_start(out=out_d[r0:r0 + tail, 0:H],
                                  in_=q_i8[0:tail, 0, :])
                nc.sync.dma_start(
                    out=out_d[r0:r0 + tail, H:H + 4].bitcast(f32),
                    in_=scl[0:tail, 0, :])

    nc.finalize()
    return nc


def _meta_key(meta):
    return (meta["T0"], meta["T1"], meta["total_bytes"],
            tuple(tuple(x) for x in meta["tw"]),
            tuple(sorted(meta["off"].items())))


_RUNTIME = {}


def _get_runtime(meta):
    key = _meta_key(meta)
    if key in _RUNTIME:
        return _RUNTIME[key]

    nc = _build_program(meta)

    import jax
    from jax.sharding import Mesh, PartitionSpec
    from jax.experimental.shard_map import shard_map
    from concourse import bass2jax

    bass2jax.install_neuronx_cc_hook()
    partition_name = nc.partition_id_tensor.name if nc.partition_id_tensor else None

    in_names, out_names, out_avals = [], [], []
    for alloc in nc.m.functions[0].allocations:
        if not isinstance(alloc, mybir.MemoryLocationSet):
            continue
        name = alloc.memorylocations[0].name
        if alloc.kind == "ExternalInput":
            if name != partition_name:
                in_names.append(name)
        elif alloc.kind == "ExternalOutput":
            out_names.append(name)
            out_avals.append(jax.core.ShapedArray(
                tuple(alloc.tensor_shape), mybir.dt.np(alloc.dtype)))
    bind_in_names = tuple(in_names) + ((partition_name,) if partition_name else ())

    def _body(*args):
        operands = list(args)
        if partition_name is not None:
            operands.append(bass2jax.partition_id_tensor())
        outs = bass2jax._bass_exec_p.bind(
            *operands, out_avals=tuple(out_avals), in_names=bind_in_names,
            out_names=tuple(out_names), lowering_input_output_aliases=(),
            sim_require_finite=True, sim_require_nnan=True, nc=nc)
        return tuple(outs)

    devices = jax.devices()[:N_CORES]
    mesh = Mesh(np.asarray(devices), ("core",))
    fn = jax.jit(
        shard_map(_body, mesh=mesh,
                  in_specs=(PartitionSpec("core"),) * len(in_names),
                  out_specs=(PartitionSpec("core"),) * len(out_names),
                  check_rep=False),
        keep_unused=True)

    rt = (fn, nc)
    _RUNTIME[key] = rt
    return rt


def _decode(out):
    """[N, 132] int8 (q | f32 scale) -> [N, 128] f32."""
    s = out[:, DIM_H:].copy().view(np.float32)
    return np.multiply(out[:, :DIM_H], s, dtype=np.float32)


_POOL = None


def _run(glob, rt):
    """One device round trip: upload blobs, execute, fetch + decode output.

    The 8 output shards are fetched on worker threads and each shard is
    dequantized as soon as it lands, so the int8 decode overlaps the
    remaining D2H transfers instead of running after them.
    """
    global _POOL
    if _POOL is None:
        from concurrent.futures import ThreadPoolExecutor
        _POOL = ThreadPoolExecutor(N_CORES)
    fn, _ = rt
    arr = fn(glob)[0]                             # [50000, 132] int8 (sharded)
    res = np.empty((N_NODES, DIM_H), np.float32)

    def grab(shard):
        r0 = shard.index[0].start or 0
        a = np.asarray(shard.data)                # D2H of this shard
        s = a[:, DIM_H:].copy().view(np.float32)
        np.multiply(a[:, :DIM_H], s, dtype=np.float32,
                    out=res[r0:r0 + a.shape[0]])

    list(_POOL.map(grab, arr.addressable_shards))
    return res


def kernel(**inputs):
    glob, meta = _host_prep(**inputs)
    rt = _get_runtime(meta)
    return _run(glob, rt)
